# revision 6
# baseline (speedup 1.0000x reference)
"""BCMSE loss kernel for 8 Trainium2 NeuronCores.

Strategy (pure data parallel, memory-bound):
  - Shard the [B, 9] inputs along batch across 8 cores (B/8 rows each).
  - Host-side, each shard is transposed to column-major [9, S] with rows
    permuted to [0,3,6,7,8,1,2,4,5] so that on device every column group
    (scalar {0,3}, vec {6,7,8}, angle {1,2,4,5}) is a contiguous block.
  - Each core streams its shard through SBUF in tiles of 128*q rows and
    reduces everything to 5 per-partition partial sums:
      p0   = sum (o-t)^2 over scalar cols
      p1   = sum wrapped-angle err^2 over angle cols
      p2   = sum (vmod-t)^2 over vec cols
      ext  = sum |floor(o)| over angle cols
      nrm  = sum norm over rows
  - Host combines 8x128 partials in float64 and applies the final formula.

Math notes:
  floor(x) = rne(x - 0.5) computed as (x + (1.5*2^23 - 0.5)) - 1.5*2^23 in
  one fused tensor_scalar op (exact except x exactly integral, measure-zero
  for randn inputs).
  angle |err| = min(|d|, ||d|-1|) with d = mod(o,1) - t  (algebraically equal
  to the reference's shortest-path target shift, incl. the |d|=0.5 boundary).
  vec remainder(v, norm) = v + norm*[v<0] exactly, because |v| <= norm.
"""
import numpy as np

import concourse.bacc as bacc
import concourse.mybir as mybir
from concourse.tile import TileContext
from concourse.bass_utils import run_bass_kernel_spmd

N_CORES = 8
BATCH = 4194304
SHARD = BATCH // N_CORES          # 524288 rows per core
P = 128
Q = 512                           # rows per partition per tile
TILE_ROWS = P * Q                 # 65536 rows per tile
N_TILES = SHARD // TILE_ROWS      # 8
PERM = [0, 3, 6, 7, 8, 1, 2, 4, 5]  # scalar(2) | vec(3) | angle(4)
MAGIC = float(1.5 * 2**23)        # rne magic for fp32
CONSTANT_WEIGHT = 10.0

_cache = {}


def _build(shard, q, n_tiles, reps=1):
    dt = mybir.dt.float32
    nc = bacc.Bacc("TRN2", target_bir_lowering=False)
    o_d = nc.dram_tensor("o", [9, shard], dt, kind="ExternalInput")
    t_d = nc.dram_tensor("t", [9, shard], dt, kind="ExternalInput")
    out_d = nc.dram_tensor("partials", [P, 8], dt, kind="ExternalOutput")

    import concourse.bass as bass

    def dram_tile(dram, i):
        # [9, shard] -> tile i as [P, 9, q]: partition stride q, block stride
        # shard, q contiguous floats per (partition, block)
        return bass.AP(dram[:].tensor, i * P * q,
                       [[q, P], [shard, 9], [1, q]])

    with TileContext(nc) as tc:
        with (
            tc.tile_pool(name="io", bufs=2) as io,
            tc.tile_pool(name="scr", bufs=1) as scr,
            tc.tile_pool(name="acc", bufs=1) as acc,
        ):
            neg1 = acc.tile([P, 1], dt, tag="neg1")
            nc.vector.memset(neg1[:], -1.0)
            negM = acc.tile([P, 1], dt, tag="negM")
            nc.vector.memset(negM[:], -MAGIC)
            s_p0 = acc.tile([P, n_tiles], dt, tag="s_p0")
            s_p1 = acc.tile([P, n_tiles], dt, tag="s_p1")
            s_p2 = acc.tile([P, n_tiles], dt, tag="s_p2")
            s_ext = acc.tile([P, n_tiles], dt, tag="s_ext")
            s_nrm = acc.tile([P, n_tiles], dt, tag="s_nrm")

            from contextlib import nullcontext
            loop = tc.For_i(0, reps, 1) if reps > 1 else nullcontext()
            with loop:
              for i in range(n_tiles):
                ot = io.tile([P, 9 * q], dt, tag="ot")
                tt = io.tile([P, 9 * q], dt, tag="tt")
                nc.sync.dma_start(out=ot[:].rearrange("p (b f) -> p b f", b=9),
                                  in_=dram_tile(o_d, i))
                nc.sync.dma_start(out=tt[:].rearrange("p (b f) -> p b f", b=9),
                                  in_=dram_tile(t_d, i))
                # contiguous column-group views (PERM order in DRAM)
                o_sc, t_sc = ot[:, 0:2 * q], tt[:, 0:2 * q]
                o_v, t_v = ot[:, 2 * q:5 * q], tt[:, 2 * q:5 * q]
                o_a, t_a = ot[:, 5 * q:9 * q], tt[:, 5 * q:9 * q]

                # ---- scalar cols: p0 += sum (o-t)^2
                pd = scr.tile([P, 2 * q], dt, tag="pd")
                nc.vector.tensor_sub(out=pd[:], in0=o_sc, in1=t_sc)
                p0o = scr.tile([P, 2 * q], dt, tag="p0o")
                nc.scalar.activation(out=p0o[:], in_=pd[:],
                                     func=mybir.ActivationFunctionType.Square,
                                     accum_out=s_p0[:, i:i + 1])

                # ---- angle cols
                # y = rne(o - 0.5) + M  (one fused tensor_scalar);
                # floor(o) = y - M, so ext accumulates |y + (-M)| via ACT bias
                y = scr.tile([P, 4 * q], dt, tag="y")
                nc.vector.tensor_scalar(out=y[:], in0=o_a,
                                        scalar1=0.5, scalar2=MAGIC,
                                        op0=mybir.AluOpType.subtract,
                                        op1=mybir.AluOpType.add)
                exto = scr.tile([P, 4 * q], dt, tag="exto")
                nc.scalar.activation(out=exto[:], in_=y[:],
                                     func=mybir.ActivationFunctionType.Abs,
                                     bias=negM[:], scale=1.0,
                                     accum_out=s_ext[:, i:i + 1])
                # negm = (y - M) - o = floor(o) - o = -mod(o, 1)
                negm = scr.tile([P, 4 * q], dt, tag="negm")
                nc.vector.scalar_tensor_tensor(out=negm[:], in0=y[:],
                                               scalar=MAGIC, in1=o_a,
                                               op0=mybir.AluOpType.subtract,
                                               op1=mybir.AluOpType.subtract)
                # s = negm + t = -(m - t) = -d;  |s| = |d|
                d = scr.tile([P, 4 * q], dt, tag="d")
                nc.vector.tensor_add(out=d[:], in0=negm[:], in1=t_a)
                ad = scr.tile([P, 4 * q], dt, tag="ad")
                nc.scalar.activation(out=ad[:], in_=d[:],
                                     func=mybir.ActivationFunctionType.Abs)
                t1 = scr.tile([P, 4 * q], dt, tag="t1")
                nc.scalar.activation(out=t1[:], in_=ad[:],
                                     func=mybir.ActivationFunctionType.Abs,
                                     bias=neg1[:], scale=1.0)
                e = scr.tile([P, 4 * q], dt, tag="e")
                nc.vector.tensor_tensor(out=e[:], in0=ad[:], in1=t1[:],
                                        op=mybir.AluOpType.min)
                p1o = scr.tile([P, 4 * q], dt, tag="p1o")
                nc.scalar.activation(out=p1o[:], in_=e[:],
                                     func=mybir.ActivationFunctionType.Square,
                                     accum_out=s_p1[:, i:i + 1])

                # ---- vec cols
                sq = scr.tile([P, 3 * q], dt, tag="sq")
                nc.scalar.activation(out=sq[:], in_=o_v,
                                     func=mybir.ActivationFunctionType.Square)
                ss1 = scr.tile([P, q], dt, tag="ss1")
                nc.vector.tensor_add(out=ss1[:], in0=sq[:, 0:q], in1=sq[:, q:2 * q])
                ss = scr.tile([P, q], dt, tag="ss")
                nc.vector.tensor_add(out=ss[:], in0=ss1[:], in1=sq[:, 2 * q:3 * q])
                nrm = scr.tile([P, q], dt, tag="nrm")
                nc.scalar.activation(out=nrm[:], in_=ss[:],
                                     func=mybir.ActivationFunctionType.Sqrt,
                                     accum_out=s_nrm[:, i:i + 1])
                w = scr.tile([P, 3 * q], dt, tag="w")
                for c in range(3):
                    nc.vector.scalar_tensor_tensor(
                        out=w[:, c * q:(c + 1) * q], in0=o_v[:, c * q:(c + 1) * q],
                        scalar=0.0, in1=nrm[:],
                        op0=mybir.AluOpType.is_lt, op1=mybir.AluOpType.mult)
                vm = scr.tile([P, 3 * q], dt, tag="vm")
                nc.vector.tensor_add(out=vm[:], in0=o_v, in1=w[:])
                dv = scr.tile([P, 3 * q], dt, tag="dv")
                nc.vector.tensor_sub(out=dv[:], in0=vm[:], in1=t_v)
                p2o = scr.tile([P, 3 * q], dt, tag="p2o")
                nc.scalar.activation(out=p2o[:], in_=dv[:],
                                     func=mybir.ActivationFunctionType.Square,
                                     accum_out=s_p2[:, i:i + 1])

            out_sb = acc.tile([P, 8], dt, tag="out_sb")
            nc.vector.memset(out_sb[:], 0.0)
            for j, s in enumerate([s_p0, s_p1, s_p2, s_ext, s_nrm]):
                nc.vector.tensor_reduce(out=out_sb[:, j:j + 1], in_=s[:],
                                        axis=mybir.AxisListType.X,
                                        op=mybir.AluOpType.add)
            nc.sync.dma_start(out=out_d[:], in_=out_sb[:])

    nc.compile()
    return nc


def _prep(arr, shard, core):
    # [B, 9] row-major -> per-core [9, shard] column-major with PERM rows
    sl = arr[core * shard:(core + 1) * shard, :]
    return np.ascontiguousarray(sl.T[PERM, :])


def _finish(partials, batch):
    # partials: [n_cores, 128, 8] fp32 -> final scalar, float64 combine
    tot = partials.astype(np.float64).sum(axis=(0, 1))
    p0, p1, p2, ext, nrm = tot[0], tot[1], tot[2], tot[3], tot[4]
    c0 = ext / batch / CONSTANT_WEIGHT
    c1 = nrm / batch / CONSTANT_WEIGHT
    mse = (p0 + p1 + p2) / (batch * 9)
    if (p0 > p1) and (p0 > p2):
        amount = 0.0
    elif (p0 > p1) and (p0 < p2):
        amount = c1
    elif (p0 < p1) and (p0 > p2):
        amount = c0
    else:
        amount = c0 + c1
    return np.float32(mse + amount)


def _run(outputs, targets, shard, q, n_tiles, n_cores, **spmd_kwargs):
    key = (shard, q, n_tiles)
    if key not in _cache:
        _cache[key] = _build(shard, q, n_tiles)
    nc = _cache[key]
    in_maps = [{"o": _prep(outputs, shard, k), "t": _prep(targets, shard, k)}
               for k in range(n_cores)]
    br = run_bass_kernel_spmd(nc, in_maps, list(range(n_cores)), **spmd_kwargs)
    partials = np.stack([r["partials"] for r in br.results])
    if spmd_kwargs:
        return partials, br
    return partials


def kernel(outputs, targets):
    outputs = np.asarray(outputs)
    targets = np.asarray(targets)
    assert outputs.shape == (BATCH, 9), outputs.shape
    partials = _run(outputs, targets, SHARD, Q, N_TILES, N_CORES)
    return _finish(partials, BATCH)


# revision 8
# speedup vs baseline: 1.0626x; 1.0626x over previous
"""BCMSE loss kernel for 8 Trainium2 NeuronCores.

Strategy (pure data parallel, memory-bound):
  - Shard the [B, 9] inputs along batch across 8 cores (B/8 rows each).
  - Host-side, each shard is transposed to column-major [9, S] with rows
    permuted to [0,3,6,7,8,1,2,4,5] so that on device every column group
    (scalar {0,3}, vec {6,7,8}, angle {1,2,4,5}) is a contiguous block.
  - Each core streams its shard through SBUF in tiles of 128*q rows and
    reduces everything to 5 per-partition partial sums:
      p0   = sum (o-t)^2 over scalar cols
      p1   = sum wrapped-angle err^2 over angle cols
      p2   = sum (vmod-t)^2 over vec cols
      ext  = sum |floor(o)| over angle cols
      nrm  = sum norm over rows
  - Host combines 8x128 partials in float64 and applies the final formula.

Math notes:
  floor(x) = rne(x - 0.5) computed as (x + (1.5*2^23 - 0.5)) - 1.5*2^23 in
  one fused tensor_scalar op (exact except x exactly integral, measure-zero
  for randn inputs).
  angle |err| = min(|d|, ||d|-1|) with d = mod(o,1) - t  (algebraically equal
  to the reference's shortest-path target shift, incl. the |d|=0.5 boundary).
  vec remainder(v, norm) = v + norm*[v<0] exactly, because |v| <= norm.
"""
import numpy as np

import concourse.bacc as bacc
import concourse.mybir as mybir
from concourse.tile import TileContext
from concourse.bass_utils import run_bass_kernel_spmd

N_CORES = 8
BATCH = 4194304
SHARD = BATCH // N_CORES          # 524288 rows per core
P = 128
Q = 512                           # rows per partition per tile
TILE_ROWS = P * Q                 # 65536 rows per tile
N_TILES = SHARD // TILE_ROWS      # 8
PERM = [0, 3, 6, 7, 8, 1, 2, 4, 5]  # scalar(2) | vec(3) | angle(4)
MAGIC = float(1.5 * 2**23)        # rne magic for fp32
CONSTANT_WEIGHT = 10.0

_cache = {}


def _build(shard, q, n_tiles, reps=1, mode='full'):
    dt = mybir.dt.float32
    nc = bacc.Bacc("TRN2", target_bir_lowering=False)
    o_d = nc.dram_tensor("o", [9, shard], dt, kind="ExternalInput")
    t_d = nc.dram_tensor("t", [9, shard], dt, kind="ExternalInput")
    out_d = nc.dram_tensor("partials", [P, 8], dt, kind="ExternalOutput")

    import concourse.bass as bass

    def dram_tile(dram, i):
        # [9, shard] -> tile i as [P, 9, q]: partition stride q, block stride
        # shard, q contiguous floats per (partition, block)
        return bass.AP(dram[:].tensor, i * P * q,
                       [[q, P], [shard, 9], [1, q]])

    with TileContext(nc) as tc:
        with (
            tc.tile_pool(name="io", bufs=2) as io,
            tc.tile_pool(name="scr", bufs=1) as scr,
            tc.tile_pool(name="acc", bufs=1) as acc,
        ):
            neg1 = acc.tile([P, 1], dt, tag="neg1")
            nc.vector.memset(neg1[:], -1.0)
            negM = acc.tile([P, 1], dt, tag="negM")
            nc.vector.memset(negM[:], -MAGIC)
            s_p0 = acc.tile([P, n_tiles], dt, tag="s_p0")
            s_p1 = acc.tile([P, n_tiles], dt, tag="s_p1")
            s_p2 = acc.tile([P, n_tiles], dt, tag="s_p2")
            s_ext = acc.tile([P, n_tiles], dt, tag="s_ext")
            s_nrm = acc.tile([P, n_tiles], dt, tag="s_nrm")
            if mode == 'dma':
                for s in (s_p0, s_p1, s_p2, s_ext, s_nrm):
                    nc.vector.memset(s[:], 0.0)

            from contextlib import nullcontext
            loop = tc.For_i(0, reps, 1) if reps > 1 else nullcontext()
            with loop:
              for i in range(n_tiles):
                ot = io.tile([P, 9 * q], dt, tag="ot")
                tt = io.tile([P, 9 * q], dt, tag="tt")
                nc.sync.dma_start(out=ot[:].rearrange("p (b f) -> p b f", b=9),
                                  in_=dram_tile(o_d, i))
                nc.sync.dma_start(out=tt[:].rearrange("p (b f) -> p b f", b=9),
                                  in_=dram_tile(t_d, i))
                if mode == 'dma':
                    continue
                # contiguous column-group views (PERM order in DRAM)
                o_sc, t_sc = ot[:, 0:2 * q], tt[:, 0:2 * q]
                o_v, t_v = ot[:, 2 * q:5 * q], tt[:, 2 * q:5 * q]
                o_a, t_a = ot[:, 5 * q:9 * q], tt[:, 5 * q:9 * q]

                # ---- scalar cols: p0 += sum (o-t)^2
                pd = scr.tile([P, 2 * q], dt, tag="pd")
                nc.vector.tensor_sub(out=pd[:], in0=o_sc, in1=t_sc)
                p0o = scr.tile([P, 2 * q], dt, tag="p0o")
                nc.scalar.activation(out=p0o[:], in_=pd[:],
                                     func=mybir.ActivationFunctionType.Square,
                                     accum_out=s_p0[:, i:i + 1])

                # ---- angle cols
                # y = rne(o - 0.5) + M  (one fused tensor_scalar);
                # floor(o) = y - M, so ext accumulates |y + (-M)| via ACT bias
                y = scr.tile([P, 4 * q], dt, tag="y")
                nc.vector.tensor_scalar(out=y[:], in0=o_a,
                                        scalar1=0.5, scalar2=MAGIC,
                                        op0=mybir.AluOpType.subtract,
                                        op1=mybir.AluOpType.add)
                exto = scr.tile([P, 4 * q], dt, tag="exto")
                nc.scalar.activation(out=exto[:], in_=y[:],
                                     func=mybir.ActivationFunctionType.Abs,
                                     bias=negM[:], scale=1.0,
                                     accum_out=s_ext[:, i:i + 1])
                # negm = (y - M) - o = floor(o) - o = -mod(o, 1)
                negm = scr.tile([P, 4 * q], dt, tag="negm")
                nc.vector.scalar_tensor_tensor(out=negm[:], in0=y[:],
                                               scalar=MAGIC, in1=o_a,
                                               op0=mybir.AluOpType.subtract,
                                               op1=mybir.AluOpType.subtract)
                # s = negm + t = -(m - t) = -d;  |s| = |d|
                d = scr.tile([P, 4 * q], dt, tag="d")
                nc.vector.tensor_add(out=d[:], in0=negm[:], in1=t_a)
                ad = scr.tile([P, 4 * q], dt, tag="ad")
                nc.scalar.activation(out=ad[:], in_=d[:],
                                     func=mybir.ActivationFunctionType.Abs)
                t1 = scr.tile([P, 4 * q], dt, tag="t1")
                nc.scalar.activation(out=t1[:], in_=ad[:],
                                     func=mybir.ActivationFunctionType.Abs,
                                     bias=neg1[:], scale=1.0)
                e = scr.tile([P, 4 * q], dt, tag="e")
                nc.vector.tensor_tensor(out=e[:], in0=ad[:], in1=t1[:],
                                        op=mybir.AluOpType.min)
                p1o = scr.tile([P, 4 * q], dt, tag="p1o")
                nc.scalar.activation(out=p1o[:], in_=e[:],
                                     func=mybir.ActivationFunctionType.Square,
                                     accum_out=s_p1[:, i:i + 1])

                # ---- vec cols
                sq = scr.tile([P, 3 * q], dt, tag="sq")
                nc.scalar.activation(out=sq[:], in_=o_v,
                                     func=mybir.ActivationFunctionType.Square)
                ss1 = scr.tile([P, q], dt, tag="ss1")
                nc.vector.tensor_add(out=ss1[:], in0=sq[:, 0:q], in1=sq[:, q:2 * q])
                ss = scr.tile([P, q], dt, tag="ss")
                nc.vector.tensor_add(out=ss[:], in0=ss1[:], in1=sq[:, 2 * q:3 * q])
                nrm = scr.tile([P, q], dt, tag="nrm")
                nc.scalar.activation(out=nrm[:], in_=ss[:],
                                     func=mybir.ActivationFunctionType.Sqrt,
                                     accum_out=s_nrm[:, i:i + 1])
                w = scr.tile([P, 3 * q], dt, tag="w")
                for c in range(3):
                    nc.vector.scalar_tensor_tensor(
                        out=w[:, c * q:(c + 1) * q], in0=o_v[:, c * q:(c + 1) * q],
                        scalar=0.0, in1=nrm[:],
                        op0=mybir.AluOpType.is_lt, op1=mybir.AluOpType.mult)
                vm = scr.tile([P, 3 * q], dt, tag="vm")
                nc.vector.tensor_add(out=vm[:], in0=o_v, in1=w[:])
                dv = scr.tile([P, 3 * q], dt, tag="dv")
                nc.vector.tensor_sub(out=dv[:], in0=vm[:], in1=t_v)
                p2o = scr.tile([P, 3 * q], dt, tag="p2o")
                nc.scalar.activation(out=p2o[:], in_=dv[:],
                                     func=mybir.ActivationFunctionType.Square,
                                     accum_out=s_p2[:, i:i + 1])

            out_sb = acc.tile([P, 8], dt, tag="out_sb")
            nc.vector.memset(out_sb[:], 0.0)
            for j, s in enumerate([s_p0, s_p1, s_p2, s_ext, s_nrm]):
                nc.vector.tensor_reduce(out=out_sb[:, j:j + 1], in_=s[:],
                                        axis=mybir.AxisListType.X,
                                        op=mybir.AluOpType.add)
            nc.sync.dma_start(out=out_d[:], in_=out_sb[:])

    nc.compile()
    return nc


def _prep(arr, shard, core):
    # [B, 9] row-major -> per-core [9, shard] column-major with PERM rows
    sl = arr[core * shard:(core + 1) * shard, :]
    return np.ascontiguousarray(sl.T[PERM, :])


def _finish(partials, batch):
    # partials: [n_cores, 128, 8] fp32 -> final scalar, float64 combine
    tot = partials.astype(np.float64).sum(axis=(0, 1))
    p0, p1, p2, ext, nrm = tot[0], tot[1], tot[2], tot[3], tot[4]
    c0 = ext / batch / CONSTANT_WEIGHT
    c1 = nrm / batch / CONSTANT_WEIGHT
    mse = (p0 + p1 + p2) / (batch * 9)
    if (p0 > p1) and (p0 > p2):
        amount = 0.0
    elif (p0 > p1) and (p0 < p2):
        amount = c1
    elif (p0 < p1) and (p0 > p2):
        amount = c0
    else:
        amount = c0 + c1
    return np.float32(mse + amount)


def _run(outputs, targets, shard, q, n_tiles, n_cores, **spmd_kwargs):
    key = (shard, q, n_tiles)
    if key not in _cache:
        _cache[key] = _build(shard, q, n_tiles)
    nc = _cache[key]
    in_maps = [{"o": _prep(outputs, shard, k), "t": _prep(targets, shard, k)}
               for k in range(n_cores)]
    br = run_bass_kernel_spmd(nc, in_maps, list(range(n_cores)), **spmd_kwargs)
    partials = np.stack([r["partials"] for r in br.results])
    if spmd_kwargs:
        return partials, br
    return partials


def kernel(outputs, targets):
    outputs = np.asarray(outputs)
    targets = np.asarray(targets)
    assert outputs.shape == (BATCH, 9), outputs.shape
    partials = _run(outputs, targets, SHARD, Q, N_TILES, N_CORES)
    return _finish(partials, BATCH)
